# revision 39
# baseline (speedup 1.0000x reference)
"""GQA attention (RoPE + ALiBi + causal) Bass kernel for Trainium2, 8 NeuronCores.

Sharding: core (b, g) = batch b in {0,1} x kv-group g in {0..3}; each core computes
its 4 query heads' attention for its batch and a partial output projection
(row-parallel wo); host sums the 4 group partials per batch.

Device dataflow:
  Phase 1 (per 512-q window): Q/K/V projections, tag-major d-loops (K,V first,
  then Q0..Q3) so RoPE (DVE, reading PSUM directly) and the V transpose overlap
  the later Q matmuls. x arrives in 4 batched DMAs per window.
  Phase 2: per (window, head): scoresT = K_u^T Q (fp32r, diagonal tiles widened
  to N>=256), P = exp(scale*scores + bias_col) in per-head-width chunks
  (local head h uses chunks of [128,256,256,512] columns; the -slope*q half of
  ALiBi cancels in softmax leaving a per-kv bias recentered per chunk), P in
  bf16; diagonal 128-blocks get a causal 0/1 bf16 mask multiply (DVE); then
  outT += V_u^T P and den += ones^T P (bf16 matmuls, fp32 PSUM accumulate);
  attn = outT * recip(den) -> bf16. Output projection (bf16) for window w is
  interleaved as PE filler during window w+1's ACT-paced attention; partials
  staged PSUM->SBUF on gpsimd and DMA'd out fp32.
"""
import math
from contextlib import ExitStack

import numpy as np
import ml_dtypes

import concourse.bass as bass
import concourse.bacc as bacc
import concourse.tile as tile
from concourse import mybir
from concourse.bass_utils import run_bass_kernel_spmd

F32 = mybir.dt.float32
F32R = mybir.dt.float32r
BF16 = mybir.dt.bfloat16
FP16 = mybir.dt.float16

B, S, D = 2, 2048, 2048
H, KV, HD, REP = 16, 4, 128, 4
NH = 4                     # heads per core
NW = S // 512              # q-windows
ND = D // 128              # d_in tiles
NU = S // 128              # kv tiles
SCALE = 1.0 / math.sqrt(HD)

# per-local-head exp chunk widths and bias recentering offsets; local head h
# within a group has the h-th largest ALiBi slope, bounding the safe chunk
# width (exp argument must stay inside fp32 range across the chunk)
EXPW = [128, 256, 256, 512]
EXPC0 = [96, 144, 144, 256]
EXPOFF = [0, 1, 1, 3]      # t128_raw offset so bias col index is 0-based


def exp_calls(w, u, h):
    """Chunk [n0,512) of window w's q-cols into exp calls for head h, tile u.
    Yields (c_lo, c_hi, bias_col). The softmax shift ref(q) must be the same
    for every kv tile of a given q row, so all tiles share the per-W_h-chunk
    recentering."""
    W = EXPW[h]
    n0 = max(0, 128 * (u - 4 * w))
    c = n0
    while c < 512:
        g_sub = c // W
        c_hi = min(512, (g_sub + 1) * W)
        g_global = (512 * w) // W + g_sub
        traw = (W // 128) * g_global - u + EXPOFF[h]
        assert 0 <= traw < 16, (w, u, h, c, traw)
        yield c, c_hi, h * 16 + traw
        c = c_hi


def build():
    nc = bacc.Bacc(None)
    xT_d = nc.dram_tensor("xT", [D, S], FP16, kind="ExternalInput")
    wq_d = nc.dram_tensor("wqT", [D, NH * HD], FP16, kind="ExternalInput")
    wkv_d = nc.dram_tensor("wkvT", [D, 2 * HD], FP16, kind="ExternalInput")
    wo_d = nc.dram_tensor("woT", [NH * HD, D], BF16, kind="ExternalInput")
    cosF_d = nc.dram_tensor("cosF", [128, S], FP16, kind="ExternalInput")
    sinF_d = nc.dram_tensor("sinF", [128, S], FP16, kind="ExternalInput")
    biasb_d = nc.dram_tensor("biasb", [128, NH * 16], F32, kind="ExternalInput")
    cmask_d = nc.dram_tensor("cmask", [128, 128], BF16, kind="ExternalInput")
    ident_d = nc.dram_tensor("ident", [128, 128], BF16, kind="ExternalInput")
    ones_d = nc.dram_tensor("ones", [128, 128], BF16, kind="ExternalInput")
    part_d = nc.dram_tensor("part", [S, D], F32, kind="ExternalOutput")

    PSUM = bass.MemorySpace.PSUM

    with tile.TileContext(nc) as tc:
        with ExitStack() as ctx:
            consts = ctx.enter_context(tc.tile_pool(name="consts", bufs=1))
            persist = ctx.enter_context(tc.tile_pool(name="persist", bufs=1))

            biasb = consts.tile([128, NH * 16], F32, tag="biasb")
            cmask = consts.tile([128, 128], BF16, tag="cmask")
            ident = consts.tile([128, 128], BF16, tag="ident")
            ones = consts.tile([128, 128], BF16, tag="ones")

            qT = [persist.tile([128, S], FP16, tag=f"qT{h}", name=f"qT{h}")
                  for h in range(NH)]
            kT = persist.tile([128, S], FP16, tag="kT")
            vT = persist.tile([128, S], BF16, tag="vT")
            vnat = persist.tile([128, S], BF16, tag="vnat")

            # ---------------- phase 1: Q/K/V projections + RoPE + V transpose
            with tc.tile_pool(name="wqkv", bufs=1) as wpool, \
                 tc.tile_pool(name="xsl", bufs=3) as xpool, \
                 tc.tile_pool(name="ctab", bufs=1) as cpool, \
                 tc.tile_pool(name="rope", bufs=4) as rp, \
                 tc.tile_pool(name="pps", bufs=1, space=PSUM) as pps:
                wq_sb = wpool.tile([128, ND, NH * HD], FP16, tag="wq")
                wkv_sb = wpool.tile([128, ND, 2 * HD], FP16, tag="wkv")
                cosF = cpool.tile([128, S], FP16, tag="cosF")
                sinF = cpool.tile([128, S], FP16, tag="sinF")
                wqr = wq_d.rearrange("(t p) o -> p t o", p=128)
                wkvr = wkv_d.rearrange("(t p) o -> p t o", p=128)
                # weights on the scalar HWDGE queue (gpsimd's software DGE is
                # ~20x slower), chunked in consumption order; tiny consts on
                # gpsimd; x + rope tables on sync, window 0 split fine
                for lo, hi in [(0, 2), (2, 4), (4, 8), (8, 12), (12, 16)]:
                    nc.scalar.dma_start(wkv_sb[:, lo:hi, :], wkvr[:, lo:hi, :])
                for q in range(4):
                    nc.scalar.dma_start(wq_sb[:, 4 * q:4 * q + 4, :],
                                        wqr[:, 4 * q:4 * q + 4, :])
                nc.gpsimd.dma_start(biasb[:], biasb_d[:])
                nc.gpsimd.dma_start(cmask[:], cmask_d[:])
                nc.gpsimd.dma_start(ident[:], ident_d[:])
                nc.gpsimd.dma_start(ones[:], ones_d[:])

                xr = xT_d.rearrange("(t p) s -> p t s", p=128)
                xs_w = []
                for w in range(NW):
                    sl = slice(w * 512, (w + 1) * 512)
                    xs = xpool.tile([128, ND, 512], FP16, tag="x", name=f"xs{w}")
                    nd_step = 2 if w == 0 else 4
                    for q in range(ND // nd_step):
                        nc.sync.dma_start(
                            xs[:, nd_step * q:nd_step * (q + 1), :],
                            xr[:, nd_step * q:nd_step * (q + 1), sl])
                    if w == 0:
                        nc.sync.dma_start(cosF[:], cosF_d[:])
                        nc.sync.dma_start(sinF[:], sinF_d[:])
                    xs_w.append(xs)

                    pq = [pps.tile([128, 512], F32, tag=f"pq{h}", name=f"pq{h}_{w}")
                          for h in range(NH)]
                    pk = pps.tile([128, 512], F32, tag="pk", name=f"pk{w}")
                    pv = pps.tile([128, 512], F32, tag="pv", name=f"pv{w}")

                    def rope(dst, src_ps):
                        # dst[:, sl] = cosF*src + sinF*swap64(src), read
                        # straight from PSUM (partition-shifted reads are
                        # legal when one operand is PSUM)
                        t1 = rp.tile([128, 512], FP16, tag="t1", name="t1")
                        qb = rp.tile([128, 512], FP16, tag="qb", name="qb")
                        nc.vector.tensor_mul(t1[:], src_ps[:], cosF[:, sl])
                        nc.vector.tensor_mul(qb[0:64, :], src_ps[64:128, :],
                                             sinF[0:64, sl])
                        nc.vector.tensor_mul(qb[64:128, :], src_ps[0:64, :],
                                             sinF[64:128, sl])
                        nc.vector.tensor_add(dst[:, sl], t1[:], qb[:])

                    # K/V first so RoPE(k) + V transpose overlap the Q d-loops
                    for d in range(ND):
                        nc.tensor.matmul(pk[:], wkv_sb[:, d, 0:HD], xs[:, d, :],
                                         start=(d == 0), stop=(d == ND - 1))
                        nc.tensor.matmul(pv[:], wkv_sb[:, d, HD:2 * HD], xs[:, d, :],
                                         start=(d == 0), stop=(d == ND - 1))
                    nc.vector.tensor_copy(vT[:, sl], pv[:])
                    rope(kT, pk)
                    for h in range(NH):
                        for d in range(ND):
                            nc.tensor.matmul(pq[h][:], wq_sb[:, d, h * 128:(h + 1) * 128],
                                             xs[:, d, :], start=(d == 0),
                                             stop=(d == ND - 1))
                        if h == 1:
                            # V transpose for this window (vT ready by now)
                            for i in range(4):
                                u = 4 * w + i
                                tp = pps.tile([128, 128], BF16, tag=f"tp{i % 2}",
                                              name=f"tp{u}")
                                nc.tensor.transpose(
                                    tp[:], vT[:, u * 128:(u + 1) * 128], ident[:])
                                nc.vector.tensor_copy(
                                    vnat[:, u * 128:(u + 1) * 128], tp[:])
                        rope(qT[h], pq[h])

            # ---------------- phase 2: attention + output projection ------------
            with tc.tile_pool(name="sp", bufs=2, space=PSUM) as sp, \
                 tc.tile_pool(name="dp", bufs=2, space=PSUM) as dp, \
                 tc.tile_pool(name="op", bufs=2, space=PSUM) as op, \
                 tc.tile_pool(name="ojp", bufs=2, space=PSUM) as ojp, \
                 tc.tile_pool(name="Pp", bufs=8) as Pp, \
                 tc.tile_pool(name="accp", bufs=2) as accpool, \
                 tc.tile_pool(name="ep", bufs=4) as ep, \
                 tc.tile_pool(name="wop", bufs=1) as wop, \
                 tc.tile_pool(name="atn", bufs=1) as apool, \
                 tc.tile_pool(name="ostg", bufs=6) as ostg:
                wo_sb = wop.tile([128, NH, D], BF16, tag="wo")
                nc.gpsimd.dma_start(wo_sb[:], wo_d.rearrange("(h p) o -> p h o", p=128))
                attn = [apool.tile([128, S], BF16, tag=f"attn{h}", name=f"attn{h}")
                        for h in range(NH)]

                filler_q = []

                def emit_fillers(n):
                    for _ in range(n):
                        if not filler_q:
                            return
                        filler_q.pop(0)()

                def make_unit(w_, mq_, dwin_):
                    def unit():
                        m_ = 4 * w_ + mq_
                        po = ojp.tile([128, 512], F32, tag="oj",
                                      name=f"po{m_}_{dwin_}")
                        for h_ in range(NH):
                            nc.tensor.matmul(
                                po[:],
                                attn[h_][:, m_ * 128:(m_ + 1) * 128],
                                wo_sb[:, h_, dwin_ * 512:(dwin_ + 1) * 512],
                                start=(h_ == 0), stop=(h_ == NH - 1))
                        so = ostg.tile([128, 512], F32, tag="so", name="so")
                        nc.vector.tensor_copy(so[:], po[:])
                        nc.sync.dma_start(
                            part_d[m_ * 128:(m_ + 1) * 128,
                                   dwin_ * 512:(dwin_ + 1) * 512], so[:])
                    return unit

                ucount = 0
                for w in range(NW):
                    qsl = slice(w * 512, (w + 1) * 512)
                    U = 4 * (w + 1)
                    for h in range(NH):
                        o_ps = op.tile([128, 512], F32, tag="o", name=f"o{w}_{h}")
                        d_ps = dp.tile([128, 512], F32, tag="den", name=f"d{w}_{h}")
                        accP = accpool.tile([128, 512], BF16, tag="aP",
                                            name=f"aP{w}_{h}")
                        pend = None
                        for u in range(U):
                            n0 = max(0, 128 * (u - 4 * w))
                            s_ps = sp.tile([128, 512], F32, tag="s", name="s")
                            nc.tensor.matmul(
                                s_ps[:, n0:512],
                                kT[:, u * 128:(u + 1) * 128],
                                qT[h][:, w * 512 + n0:(w + 1) * 512],
                                start=True, stop=True)
                            Pt = Pp.tile([128, 512], BF16, tag="P", name="P")
                            for c_lo, c_hi, bcol in exp_calls(w, u, h):
                                nc.scalar.activation(
                                    Pt[:, c_lo:c_hi], s_ps[:, c_lo:c_hi],
                                    mybir.ActivationFunctionType.Exp,
                                    bias=biasb[:, bcol:bcol + 1],
                                    scale=SCALE)
                            if u >= 4 * w:
                                nc.vector.tensor_mul(
                                    Pt[:, n0:n0 + 128], Pt[:, n0:n0 + 128],
                                    cmask[:])
                            # running sum of P on DVE; den = ones^T sum (one
                            # matmul per (w,h) instead of one per tile)
                            if u == 0:
                                nc.vector.tensor_copy(accP[:], Pt[:])
                            else:
                                nc.vector.tensor_add(accP[:, n0:512],
                                                     accP[:, n0:512],
                                                     Pt[:, n0:512])
                            ucount += 1
                            if pend is not None:
                                pPt, pn0, pu = pend
                                nc.tensor.matmul(o_ps[:, pn0:512],
                                                 vnat[:, pu * 128:(pu + 1) * 128],
                                                 pPt[:, pn0:512],
                                                 start=(pu == 0), stop=False)
                            pend = (Pt, n0, u)
                            # filler after PV, not between scores and PV: the
                            # scores->exp->PV chain must not queue behind it
                            if ucount % 2 == 0 or h == 0:
                                emit_fillers(1)
                        pPt, pn0, pu = pend
                        nc.tensor.matmul(o_ps[:, pn0:512],
                                         vnat[:, pu * 128:(pu + 1) * 128],
                                         pPt[:, pn0:512], start=(pu == 0), stop=True)
                        nc.tensor.matmul(d_ps[:], ones[:], accP[:],
                                         start=True, stop=True)
                        rec = ep.tile([128, 512], F32, tag="rec", name="rec")
                        nc.vector.reciprocal_approx_fast(rec[:], d_ps[:])
                        nc.vector.tensor_mul(attn[h][:, qsl], o_ps[:], rec[:])

                    # enqueue this window's output projection as PE filler
                    # for the next window's ACT-paced attention loop
                    for mq in range(4):
                        for dwin in range(4):
                            filler_q.append(make_unit(w, mq, dwin))

                emit_fillers(len(filler_q))
    nc.finalize()
    return nc


_NC_CACHE = {}


def _get_nc():
    if "nc" not in _NC_CACHE:
        _NC_CACHE["nc"] = build()
    return _NC_CACHE["nc"]


def _host_prep(x, alibi_bias, wq, wk, wv, wo):
    """Build per-core input maps (shard + transpose + rope tables + bias tables)."""
    x = np.asarray(x, np.float32)
    alibi_bias = np.asarray(alibi_bias, np.float32)
    wq = np.asarray(wq, np.float32)
    wk = np.asarray(wk, np.float32)
    wv = np.asarray(wv, np.float32)
    wo = np.asarray(wo, np.float32)
    BF = ml_dtypes.bfloat16

    slopes = alibi_bias[0, :, 0, 1].copy()        # [H]; alibi[0,h,0,1] = slope_h

    inv_freq = 1.0 / (10000.0 ** (np.arange(0, HD, 2, dtype=np.float32) / HD))
    t = np.arange(S, dtype=np.float32)
    freqs = np.outer(t, inv_freq)                 # [S, 64]
    cos = np.cos(freqs).astype(np.float32).T      # [64, S]
    sin = np.sin(freqs).astype(np.float32).T
    cosF = np.ascontiguousarray(np.concatenate([cos, cos], 0)).astype(np.float16)
    sinF = np.ascontiguousarray(np.concatenate([-sin, sin], 0)).astype(np.float16)

    perm = np.concatenate([np.arange(0, HD, 2), np.arange(1, HD, 2)])
    p_ar = np.arange(128, dtype=np.float32)
    cmask = (p_ar[:, None] <= p_ar[None, :]).astype(BF)
    ident = np.eye(128, dtype=BF)
    ones = np.ones((128, 128), BF)

    xTs = [np.ascontiguousarray(x[b].T).astype(np.float16) for b in range(B)]
    in_maps = []
    for core in range(8):
        b, g = divmod(core, KV)
        wq_g = wq[4 * g * HD:(4 * g + 4) * HD].reshape(NH, HD, D)[:, perm, :]
        wqT = np.ascontiguousarray(wq_g.reshape(NH * HD, D).T).astype(np.float16)
        wkvT = np.ascontiguousarray(np.concatenate(
            [wk[g * HD:(g + 1) * HD][perm], wv[g * HD:(g + 1) * HD]], 0
        ).T).astype(np.float16)
        woT = np.ascontiguousarray(wo[:, 4 * g * HD:(4 * g + 4) * HD].T).astype(BF)
        biasb = np.zeros((128, NH * 16), np.float32)
        for h in range(NH):
            sl = slopes[4 * g + h]
            for tt in range(16):
                traw = tt - EXPOFF[h]
                biasb[:, h * 16 + tt] = np.maximum(
                    sl * (p_ar - EXPC0[h] - 128.0 * traw), -200.0)
        in_maps.append({
            "xT": xTs[b], "wqT": wqT, "wkvT": wkvT, "woT": woT,
            "cosF": cosF, "sinF": sinF, "biasb": biasb, "cmask": cmask,
            "ident": ident, "ones": ones,
        })
    return in_maps


def kernel(x, mask, alibi_bias, wq, wk, wv, wo, _trace=False, _trace_kwargs=None):
    nc = _get_nc()
    in_maps = _host_prep(x, alibi_bias, wq, wk, wv, wo)
    res = run_bass_kernel_spmd(nc, in_maps, list(range(8)), trace=_trace,
                               **(_trace_kwargs or {}))
    parts = [res.results[c]["part"] for c in range(8)]
    out = np.stack([
        parts[0] + parts[1] + parts[2] + parts[3],
        parts[4] + parts[5] + parts[6] + parts[7],
    ]).astype(np.float32)
    if _trace:
        return out, res
    return out


# revision 40
# speedup vs baseline: 1.1770x; 1.1770x over previous
"""GQA attention (RoPE + ALiBi + causal) Bass kernel for Trainium2, 8 NeuronCores.

Sharding: core (b, g) = batch b in {0,1} x kv-group g in {0..3}; each core computes
its 4 query heads' attention for its batch and a partial output projection
(row-parallel wo); host sums the 4 group partials per batch.

Device dataflow:
  Phase 1 (per 512-q window): Q/K/V projections, tag-major d-loops (K,V first,
  then Q0..Q3) so RoPE (DVE, reading PSUM directly) and the V transpose overlap
  the later Q matmuls. x arrives in 4 batched DMAs per window.
  Phase 2: per (window, head): scoresT = K_u^T Q (fp32r, diagonal tiles widened
  to N>=256), P = exp(scale*scores + bias_col) in per-head-width chunks
  (local head h uses chunks of [128,256,256,512] columns; the -slope*q half of
  ALiBi cancels in softmax leaving a per-kv bias recentered per chunk), P in
  bf16; diagonal 128-blocks get a causal 0/1 bf16 mask multiply (DVE); then
  outT += V_u^T P and den += ones^T P (bf16 matmuls, fp32 PSUM accumulate);
  attn = outT * recip(den) -> bf16. Output projection (bf16) for window w is
  interleaved as PE filler during window w+1's ACT-paced attention; partials
  staged PSUM->SBUF on gpsimd and DMA'd out fp32.
"""
import math
from contextlib import ExitStack

import numpy as np
import ml_dtypes

import concourse.bass as bass
import concourse.bacc as bacc
import concourse.tile as tile
from concourse import mybir
from concourse.bass_utils import run_bass_kernel_spmd

F32 = mybir.dt.float32
F32R = mybir.dt.float32r
BF16 = mybir.dt.bfloat16
FP16 = mybir.dt.float16

B, S, D = 2, 2048, 2048
H, KV, HD, REP = 16, 4, 128, 4
NH = 4                     # heads per core
NW = S // 512              # q-windows
ND = D // 128              # d_in tiles
NU = S // 128              # kv tiles
SCALE = 1.0 / math.sqrt(HD)

# per-local-head exp chunk widths and bias recentering offsets; local head h
# within a group has the h-th largest ALiBi slope, bounding the safe chunk
# width (exp argument must stay inside fp32 range across the chunk)
EXPW = [128, 256, 256, 512]
EXPC0 = [96, 144, 144, 256]
EXPOFF = [0, 1, 1, 3]      # t128_raw offset so bias col index is 0-based


def exp_calls(w, u, h):
    """Chunk [n0,512) of window w's q-cols into exp calls for head h, tile u.
    Yields (c_lo, c_hi, bias_col). The softmax shift ref(q) must be the same
    for every kv tile of a given q row, so all tiles share the per-W_h-chunk
    recentering."""
    W = EXPW[h]
    n0 = max(0, 128 * (u - 4 * w))
    c = n0
    while c < 512:
        g_sub = c // W
        c_hi = min(512, (g_sub + 1) * W)
        g_global = (512 * w) // W + g_sub
        traw = (W // 128) * g_global - u + EXPOFF[h]
        assert 0 <= traw < 16, (w, u, h, c, traw)
        yield c, c_hi, h * 16 + traw
        c = c_hi


def build():
    nc = bacc.Bacc(None)
    xT_d = nc.dram_tensor("xT", [D, S], FP16, kind="ExternalInput")
    wq_d = nc.dram_tensor("wqT", [D, NH * HD], FP16, kind="ExternalInput")
    wkv_d = nc.dram_tensor("wkvT", [D, 2 * HD], FP16, kind="ExternalInput")
    wo_d = nc.dram_tensor("woT", [NH * HD, D], BF16, kind="ExternalInput")
    cosF_d = nc.dram_tensor("cosF", [128, S], FP16, kind="ExternalInput")
    sinF_d = nc.dram_tensor("sinF", [128, S], FP16, kind="ExternalInput")
    biasb_d = nc.dram_tensor("biasb", [128, NH * 16], F32, kind="ExternalInput")
    cmask_d = nc.dram_tensor("cmask", [128, 128], BF16, kind="ExternalInput")
    ident_d = nc.dram_tensor("ident", [128, 128], BF16, kind="ExternalInput")
    ones_d = nc.dram_tensor("ones", [128, 128], BF16, kind="ExternalInput")
    part_d = nc.dram_tensor("part", [S, D], F32, kind="ExternalOutput")

    PSUM = bass.MemorySpace.PSUM

    with tile.TileContext(nc) as tc:
        with ExitStack() as ctx:
            consts = ctx.enter_context(tc.tile_pool(name="consts", bufs=1))
            persist = ctx.enter_context(tc.tile_pool(name="persist", bufs=1))

            biasb = consts.tile([128, NH * 16], F32, tag="biasb")
            cmask = consts.tile([128, 128], BF16, tag="cmask")
            ident = consts.tile([128, 128], BF16, tag="ident")
            ones = consts.tile([128, 128], BF16, tag="ones")

            qT = [persist.tile([128, S], FP16, tag=f"qT{h}", name=f"qT{h}")
                  for h in range(NH)]
            kT = persist.tile([128, S], FP16, tag="kT")
            vT = persist.tile([128, S], BF16, tag="vT")
            vnat = persist.tile([128, S], BF16, tag="vnat")

            # ---------------- phase 1: Q/K/V projections + RoPE + V transpose
            with tc.tile_pool(name="wqkv", bufs=1) as wpool, \
                 tc.tile_pool(name="xsl", bufs=3) as xpool, \
                 tc.tile_pool(name="ctab", bufs=1) as cpool, \
                 tc.tile_pool(name="rope", bufs=4) as rp, \
                 tc.tile_pool(name="pps", bufs=1, space=PSUM) as pps:
                wq_sb = wpool.tile([128, ND, NH * HD], FP16, tag="wq")
                wkv_sb = wpool.tile([128, ND, 2 * HD], FP16, tag="wkv")
                cosF = cpool.tile([128, S], FP16, tag="cosF")
                sinF = cpool.tile([128, S], FP16, tag="sinF")
                wqr = wq_d.rearrange("(t p) o -> p t o", p=128)
                wkvr = wkv_d.rearrange("(t p) o -> p t o", p=128)
                # weights on the scalar HWDGE queue (gpsimd's software DGE is
                # ~20x slower), chunked in consumption order; tiny consts on
                # gpsimd; x + rope tables on sync, window 0 split fine
                for lo, hi in [(0, 2), (2, 4), (4, 8), (8, 12), (12, 16)]:
                    nc.scalar.dma_start(wkv_sb[:, lo:hi, :], wkvr[:, lo:hi, :])
                for q in range(4):
                    nc.scalar.dma_start(wq_sb[:, 4 * q:4 * q + 4, :],
                                        wqr[:, 4 * q:4 * q + 4, :])
                nc.gpsimd.dma_start(biasb[:], biasb_d[:])
                nc.gpsimd.dma_start(cmask[:], cmask_d[:])
                nc.gpsimd.dma_start(ident[:], ident_d[:])
                nc.gpsimd.dma_start(ones[:], ones_d[:])

                xr = xT_d.rearrange("(t p) s -> p t s", p=128)
                xs_w = []
                for w in range(NW):
                    sl = slice(w * 512, (w + 1) * 512)
                    xs = xpool.tile([128, ND, 512], FP16, tag="x", name=f"xs{w}")
                    nd_step = 2 if w == 0 else 4
                    for q in range(ND // nd_step):
                        nc.sync.dma_start(
                            xs[:, nd_step * q:nd_step * (q + 1), :],
                            xr[:, nd_step * q:nd_step * (q + 1), sl])
                    if w == 0:
                        nc.sync.dma_start(cosF[:], cosF_d[:])
                        nc.sync.dma_start(sinF[:], sinF_d[:])
                    xs_w.append(xs)

                    pq = [pps.tile([128, 512], F32, tag=f"pq{h}", name=f"pq{h}_{w}")
                          for h in range(NH)]
                    pk = pps.tile([128, 512], F32, tag="pk", name=f"pk{w}")
                    pv = pps.tile([128, 512], F32, tag="pv", name=f"pv{w}")

                    def rope(dst, src_ps):
                        # dst[:, sl] = cosF*src + sinF*swap64(src), read
                        # straight from PSUM (partition-shifted reads are
                        # legal when one operand is PSUM)
                        t1 = rp.tile([128, 512], FP16, tag="t1", name="t1")
                        qb = rp.tile([128, 512], FP16, tag="qb", name="qb")
                        nc.vector.tensor_mul(t1[:], src_ps[:], cosF[:, sl])
                        nc.vector.tensor_mul(qb[0:64, :], src_ps[64:128, :],
                                             sinF[0:64, sl])
                        nc.vector.tensor_mul(qb[64:128, :], src_ps[0:64, :],
                                             sinF[64:128, sl])
                        nc.vector.tensor_add(dst[:, sl], t1[:], qb[:])

                    # K/V first so RoPE(k) + V transpose overlap the Q d-loops
                    for d in range(ND):
                        nc.tensor.matmul(pk[:], wkv_sb[:, d, 0:HD], xs[:, d, :],
                                         start=(d == 0), stop=(d == ND - 1))
                        nc.tensor.matmul(pv[:], wkv_sb[:, d, HD:2 * HD], xs[:, d, :],
                                         start=(d == 0), stop=(d == ND - 1))
                    nc.vector.tensor_copy(vT[:, sl], pv[:])
                    rope(kT, pk)
                    for h in range(NH):
                        for d in range(ND):
                            nc.tensor.matmul(pq[h][:], wq_sb[:, d, h * 128:(h + 1) * 128],
                                             xs[:, d, :], start=(d == 0),
                                             stop=(d == ND - 1))
                        if h == 1:
                            # V transpose for this window (vT ready by now)
                            for i in range(4):
                                u = 4 * w + i
                                tp = pps.tile([128, 128], BF16, tag=f"tp{i % 2}",
                                              name=f"tp{u}")
                                nc.tensor.transpose(
                                    tp[:], vT[:, u * 128:(u + 1) * 128], ident[:])
                                nc.vector.tensor_copy(
                                    vnat[:, u * 128:(u + 1) * 128], tp[:])
                        rope(qT[h], pq[h])

            # ---------------- phase 2: attention + output projection ------------
            with tc.tile_pool(name="sp", bufs=2, space=PSUM) as sp, \
                 tc.tile_pool(name="dp", bufs=2, space=PSUM) as dp, \
                 tc.tile_pool(name="op", bufs=2, space=PSUM) as op, \
                 tc.tile_pool(name="ojp", bufs=2, space=PSUM) as ojp, \
                 tc.tile_pool(name="Pp", bufs=8) as Pp, \
                 tc.tile_pool(name="accp", bufs=2) as accpool, \
                 tc.tile_pool(name="ep", bufs=4) as ep, \
                 tc.tile_pool(name="wop", bufs=1) as wop, \
                 tc.tile_pool(name="atn", bufs=1) as apool, \
                 tc.tile_pool(name="ostg", bufs=6) as ostg:
                wo_sb = wop.tile([128, NH, D], BF16, tag="wo")
                nc.gpsimd.dma_start(wo_sb[:], wo_d.rearrange("(h p) o -> p h o", p=128))
                attn = [apool.tile([128, S], BF16, tag=f"attn{h}", name=f"attn{h}")
                        for h in range(NH)]

                filler_q = []

                def emit_fillers(n):
                    for _ in range(n):
                        if not filler_q:
                            return
                        filler_q.pop(0)()

                def make_unit(w_, mq_, dwin_):
                    def unit():
                        m_ = 4 * w_ + mq_
                        po = ojp.tile([128, 512], F32, tag="oj",
                                      name=f"po{m_}_{dwin_}")
                        for h_ in range(NH):
                            nc.tensor.matmul(
                                po[:],
                                attn[h_][:, m_ * 128:(m_ + 1) * 128],
                                wo_sb[:, h_, dwin_ * 512:(dwin_ + 1) * 512],
                                start=(h_ == 0), stop=(h_ == NH - 1))
                        so = ostg.tile([128, 512], F32, tag="so", name="so")
                        nc.vector.tensor_copy(so[:], po[:])
                        nc.sync.dma_start(
                            part_d[m_ * 128:(m_ + 1) * 128,
                                   dwin_ * 512:(dwin_ + 1) * 512], so[:])
                    return unit

                ucount = 0
                for w in range(NW):
                    qsl = slice(w * 512, (w + 1) * 512)
                    U = 4 * (w + 1)
                    for h in range(NH):
                        o_ps = op.tile([128, 512], F32, tag="o", name=f"o{w}_{h}")
                        d_ps = dp.tile([128, 512], F32, tag="den", name=f"d{w}_{h}")
                        accP = accpool.tile([128, 512], BF16, tag="aP",
                                            name=f"aP{w}_{h}")
                        pend = None
                        for u in range(U):
                            n0 = max(0, 128 * (u - 4 * w))
                            s_ps = sp.tile([128, 512], F32, tag="s", name="s")
                            nc.tensor.matmul(
                                s_ps[:, n0:512],
                                kT[:, u * 128:(u + 1) * 128],
                                qT[h][:, w * 512 + n0:(w + 1) * 512],
                                start=True, stop=True)
                            Pt = Pp.tile([128, 512], BF16, tag="P", name="P")
                            for c_lo, c_hi, bcol in exp_calls(w, u, h):
                                nc.scalar.activation(
                                    Pt[:, c_lo:c_hi], s_ps[:, c_lo:c_hi],
                                    mybir.ActivationFunctionType.Exp,
                                    bias=biasb[:, bcol:bcol + 1],
                                    scale=SCALE)
                            if u >= 4 * w:
                                nc.vector.tensor_mul(
                                    Pt[:, n0:n0 + 128], Pt[:, n0:n0 + 128],
                                    cmask[:])
                            # running sum of P on DVE; den = ones^T sum (one
                            # matmul per (w,h) instead of one per tile)
                            if u == 0:
                                nc.vector.tensor_copy(accP[:], Pt[:])
                            else:
                                nc.vector.tensor_add(accP[:, n0:512],
                                                     accP[:, n0:512],
                                                     Pt[:, n0:512])
                            ucount += 1
                            if ucount % 2 == 0 or h == 0:
                                emit_fillers(1)
                            if pend is not None:
                                pPt, pn0, pu = pend
                                nc.tensor.matmul(o_ps[:, pn0:512],
                                                 vnat[:, pu * 128:(pu + 1) * 128],
                                                 pPt[:, pn0:512],
                                                 start=(pu == 0), stop=False)
                            pend = (Pt, n0, u)
                        pPt, pn0, pu = pend
                        nc.tensor.matmul(o_ps[:, pn0:512],
                                         vnat[:, pu * 128:(pu + 1) * 128],
                                         pPt[:, pn0:512], start=(pu == 0), stop=True)
                        nc.tensor.matmul(d_ps[:], ones[:], accP[:],
                                         start=True, stop=True)
                        rec = ep.tile([128, 512], F32, tag="rec", name="rec")
                        nc.vector.reciprocal_approx_fast(rec[:], d_ps[:])
                        nc.vector.tensor_mul(attn[h][:, qsl], o_ps[:], rec[:])

                    # enqueue this window's output projection as PE filler
                    # for the next window's ACT-paced attention loop
                    for mq in range(4):
                        for dwin in range(4):
                            filler_q.append(make_unit(w, mq, dwin))

                emit_fillers(len(filler_q))
    nc.finalize()
    return nc


_NC_CACHE = {}


def _get_nc():
    if "nc" not in _NC_CACHE:
        _NC_CACHE["nc"] = build()
    return _NC_CACHE["nc"]


def _host_prep(x, alibi_bias, wq, wk, wv, wo):
    """Build per-core input maps (shard + transpose + rope tables + bias tables)."""
    x = np.asarray(x, np.float32)
    alibi_bias = np.asarray(alibi_bias, np.float32)
    wq = np.asarray(wq, np.float32)
    wk = np.asarray(wk, np.float32)
    wv = np.asarray(wv, np.float32)
    wo = np.asarray(wo, np.float32)
    BF = ml_dtypes.bfloat16

    slopes = alibi_bias[0, :, 0, 1].copy()        # [H]; alibi[0,h,0,1] = slope_h

    inv_freq = 1.0 / (10000.0 ** (np.arange(0, HD, 2, dtype=np.float32) / HD))
    t = np.arange(S, dtype=np.float32)
    freqs = np.outer(t, inv_freq)                 # [S, 64]
    cos = np.cos(freqs).astype(np.float32).T      # [64, S]
    sin = np.sin(freqs).astype(np.float32).T
    cosF = np.ascontiguousarray(np.concatenate([cos, cos], 0)).astype(np.float16)
    sinF = np.ascontiguousarray(np.concatenate([-sin, sin], 0)).astype(np.float16)

    perm = np.concatenate([np.arange(0, HD, 2), np.arange(1, HD, 2)])
    p_ar = np.arange(128, dtype=np.float32)
    cmask = (p_ar[:, None] <= p_ar[None, :]).astype(BF)
    ident = np.eye(128, dtype=BF)
    ones = np.ones((128, 128), BF)

    xTs = [np.ascontiguousarray(x[b].T).astype(np.float16) for b in range(B)]
    in_maps = []
    for core in range(8):
        b, g = divmod(core, KV)
        wq_g = wq[4 * g * HD:(4 * g + 4) * HD].reshape(NH, HD, D)[:, perm, :]
        wqT = np.ascontiguousarray(wq_g.reshape(NH * HD, D).T).astype(np.float16)
        wkvT = np.ascontiguousarray(np.concatenate(
            [wk[g * HD:(g + 1) * HD][perm], wv[g * HD:(g + 1) * HD]], 0
        ).T).astype(np.float16)
        woT = np.ascontiguousarray(wo[:, 4 * g * HD:(4 * g + 4) * HD].T).astype(BF)
        biasb = np.zeros((128, NH * 16), np.float32)
        for h in range(NH):
            sl = slopes[4 * g + h]
            for tt in range(16):
                traw = tt - EXPOFF[h]
                biasb[:, h * 16 + tt] = np.maximum(
                    sl * (p_ar - EXPC0[h] - 128.0 * traw), -200.0)
        in_maps.append({
            "xT": xTs[b], "wqT": wqT, "wkvT": wkvT, "woT": woT,
            "cosF": cosF, "sinF": sinF, "biasb": biasb, "cmask": cmask,
            "ident": ident, "ones": ones,
        })
    return in_maps


def kernel(x, mask, alibi_bias, wq, wk, wv, wo, _trace=False, _trace_kwargs=None):
    nc = _get_nc()
    in_maps = _host_prep(x, alibi_bias, wq, wk, wv, wo)
    res = run_bass_kernel_spmd(nc, in_maps, list(range(8)), trace=_trace,
                               **(_trace_kwargs or {}))
    parts = [res.results[c]["part"] for c in range(8)]
    out = np.stack([
        parts[0] + parts[1] + parts[2] + parts[3],
        parts[4] + parts[5] + parts[6] + parts[7],
    ]).astype(np.float32)
    if _trace:
        return out, res
    return out
